# revision 4
# baseline (speedup 1.0000x reference)
"""CopyNet additive-attention kernel for 8 Trainium2 NeuronCores.

Problem (B=4, S=T=512, D=256):
    src_lin = source @ W_src + b_src                    (B,S,D)
    tgt_lin = target @ W_tgt + b_tgt                    (B,T,D)
    genP    = tanh(src_lin[:,None] + tgt_lin[:,:,None]) @ W_res[:,0] + b_res
    prob    = softmax(target @ W_prob + b_prob)         (B,T,2)

The O(B*T*S*D) = 268M-element tanh + weighted reduction runs on device; the
tiny linear layers / prob head (0.2% of the FLOPs) are host-side prep.

Sharding: 8 cores = 4 batches x 2 target-halves. Each core handles one
(b, 256-target-slice) shard: 33.5M tanh evaluations.

Per-core device program (D on partitions as 2x128 chunks, S on free axis):
  - DVE  : broadcast-add tgt_lin[:,t] (per-partition scalar, 2x mode)
  - ACT  : tanh over G-target batches (amortizes the ~224-cycle instr overhead)
  - PE   : float32r matmuls with a shifted-diagonal weight tile so each
           target's D-reduction accumulates into its own PSUM partition row
ACT is the bottleneck engine at ~1 elem/cycle/lane/1.2GHz.
"""

import numpy as np
from contextlib import ExitStack

import concourse.bass as bass
import concourse.tile as tile
from concourse import bacc, mybir
from concourse import bass_utils

F32 = mybir.dt.float32
F32R = mybir.dt.float32r
AF = mybir.ActivationFunctionType

# Hardcoded problem geometry (per-core shard)
D = 256      # feature dim (2 partition chunks of 128)
S = 512      # source positions (free axis)
TC = 256     # target positions per core
NCORES = 8
G = 8        # targets per ACT batch

_nc_cache = None


def build_nc():
    nc = bacc.Bacc("TRN2", target_bir_lowering=False, debug=False)

    slt = nc.dram_tensor("slt", [D, S], F32, kind="ExternalInput").ap()
    tlt = nc.dram_tensor("tlt", [D, TC], F32, kind="ExternalInput").ap()
    wbig = nc.dram_tensor("wbig", [D, 255], F32, kind="ExternalInput").ap()
    bres = nc.dram_tensor("bres", [128, 1], F32, kind="ExternalInput").ap()
    genp = nc.dram_tensor("genp", [TC, S], F32, kind="ExternalOutput").ap()

    with tile.TileContext(nc) as tc, ExitStack() as ctx:
        const = ctx.enter_context(tc.tile_pool(name="const", bufs=1))
        work = ctx.enter_context(tc.tile_pool(name="work", bufs=2))
        outp = ctx.enter_context(tc.tile_pool(name="outp", bufs=2))
        psump = ctx.enter_context(
            tc.tile_pool(name="psum", bufs=2, space="PSUM")
        )

        sltt, tltt, wt = [], [], []
        for dc in range(2):
            t1 = const.tile([128, S], F32, tag=f"slt{dc}")
            nc.sync.dma_start(out=t1[:], in_=slt[dc * 128:(dc + 1) * 128, :])
            sltt.append(t1)
            t2 = const.tile([128, TC], F32, tag=f"tlt{dc}")
            nc.sync.dma_start(out=t2[:], in_=tlt[dc * 128:(dc + 1) * 128, :])
            tltt.append(t2)
            t3s = const.tile([128, 255], F32, tag=f"wstage{dc}")
            nc.sync.dma_start(out=t3s[:], in_=wbig[dc * 128:(dc + 1) * 128, :])
            t3 = const.tile([128, 255], F32R, tag=f"w{dc}")
            nc.vector.tensor_copy(t3[:], t3s[:])
            wt.append(t3)
        bt = const.tile([128, 1], F32, tag="bres")
        nc.sync.dma_start(out=bt[:], in_=bres[:, :])

        n_blocks = TC // 128
        n_groups = 128 // G
        for tb in range(n_blocks):
            ps = psump.tile([128, S], F32)
            first = True
            for g in range(n_groups):
                hb = []
                for dc in range(2):
                    sm = work.tile([128, G * S], F32, tag=f"sum{dc}")
                    for j in range(G):
                        t = tb * 128 + g * G + j
                        nc.vector.tensor_scalar_add(
                            sm[:, j * S:(j + 1) * S],
                            sltt[dc][:],
                            tltt[dc][:, t:t + 1],
                        )
                    h = work.tile([128, G * S], F32R, tag=f"h{dc}")
                    nc.scalar.activation(h[:], sm[:], AF.Tanh)
                    hb.append(h)
                for j in range(G):
                    tl = g * G + j  # local row in this psum block
                    for dc in range(2):
                        last = (g == n_groups - 1) and (j == G - 1) and (dc == 1)
                        nc.tensor.matmul(
                            ps[:],
                            lhsT=wt[dc][:, 127 - tl:255 - tl],
                            rhs=hb[dc][:, j * S:(j + 1) * S],
                            start=first,
                            stop=last,
                        )
                        first = False
            ob = outp.tile([128, S], F32)
            nc.vector.tensor_scalar_add(ob[:], ps[:], bt[:, 0:1])
            nc.sync.dma_start(
                out=genp[tb * 128:(tb + 1) * 128, :], in_=ob[:]
            )

    nc.compile()
    return nc


def _get_nc():
    global _nc_cache
    if _nc_cache is None:
        _nc_cache = build_nc()
    return _nc_cache


def _host_prep(source, target, W_src, b_src, W_tgt, b_tgt, W_res, b_res):
    B = source.shape[0]
    T = target.shape[1]
    src_lin = source.reshape(B * S, D).astype(np.float32) @ W_src + b_src
    tgt_lin = target.reshape(B * T, D).astype(np.float32) @ W_tgt + b_tgt
    src_linT = np.ascontiguousarray(
        src_lin.reshape(B, S, D).transpose(0, 2, 1)
    )  # (B, D, S)
    tgt_linT = np.ascontiguousarray(
        tgt_lin.reshape(B, T, D).transpose(0, 2, 1)
    )  # (B, D, T)
    wbig = np.zeros((D, 255), np.float32)
    wbig[:, 127] = W_res[:, 0]
    bres = np.full((128, 1), np.float32(b_res[0]), np.float32)
    in_maps = []
    for c in range(NCORES):
        b, th = c // 2, c % 2
        in_maps.append({
            "slt": src_linT[b],
            "tlt": np.ascontiguousarray(
                tgt_linT[b, :, th * TC:(th + 1) * TC]
            ),
            "wbig": wbig,
            "bres": bres,
        })
    return in_maps


def kernel(source, target, W_src, b_src, W_tgt, b_tgt, W_res, b_res,
           W_prob, b_prob, _trace=False):
    source = np.asarray(source, np.float32)
    target = np.asarray(target, np.float32)
    W_src = np.asarray(W_src, np.float32)
    b_src = np.asarray(b_src, np.float32)
    W_tgt = np.asarray(W_tgt, np.float32)
    b_tgt = np.asarray(b_tgt, np.float32)
    W_res = np.asarray(W_res, np.float32)
    b_res = np.asarray(b_res, np.float32)
    W_prob = np.asarray(W_prob, np.float32)
    b_prob = np.asarray(b_prob, np.float32)

    B = source.shape[0]
    T = target.shape[1]

    in_maps = _host_prep(source, target, W_src, b_src, W_tgt, b_tgt,
                         W_res, b_res)
    nc = _get_nc()
    res = bass_utils.run_bass_kernel_spmd(
        nc, in_maps, list(range(NCORES)), trace=_trace
    )

    genP = np.empty((B, T, S), np.float32)
    for c in range(NCORES):
        b, th = c // 2, c % 2
        genP[b, th * TC:(th + 1) * TC, :] = res.results[c]["genp"]

    logits = target.reshape(B * T, D) @ W_prob + b_prob
    m = logits.max(axis=-1, keepdims=True)
    e = np.exp(logits - m)
    prob = (e / e.sum(axis=-1, keepdims=True)).reshape(B, T, 2)
    prob = prob.astype(np.float32)

    if _trace:
        kernel._last_result = res
    return genP, prob


# revision 8
# speedup vs baseline: 1.0772x; 1.0772x over previous
"""CopyNet additive-attention kernel for 8 Trainium2 NeuronCores.

Problem (B=4, S=T=512, D=256):
    src_lin = source @ W_src + b_src                    (B,S,D)
    tgt_lin = target @ W_tgt + b_tgt                    (B,T,D)
    genP    = tanh(src_lin[:,None] + tgt_lin[:,:,None]) @ W_res[:,0] + b_res
    prob    = softmax(target @ W_prob + b_prob)         (B,T,2)

The O(B*T*S*D) = 268M-element tanh + weighted reduction runs on device; the
tiny linear layers / prob head (0.2% of the FLOPs) are host-side prep.

Sharding: 8 cores = 4 batches x 2 target-halves. Each core handles one
(b, 256-target-slice) shard: 33.5M tanh evaluations.

Per-core device program (D on partitions as 2x128 chunks, S on free axis):
  - DVE+GPSIMD: broadcast-add tgt_lin[:,t] (per-partition scalar), split
    between the two engines so neither is the bottleneck
  - ACT: tanh over 16-target batches (amortizes per-instr overhead),
    fp32 in -> fp16 out
  - PE: fp16 matmuls (1 cyc/row) with a shifted-diagonal weight tile so each
    target's D-reduction accumulates into its own PSUM partition row; b_res
    enters as a K=1 rank-1 matmul; output DMAs straight from PSUM
ACT is the bottleneck engine at ~1 elem/cycle/lane/1.2GHz -> ~224us/core.
"""

import os
import numpy as np
from contextlib import ExitStack

import concourse.bass as bass
import concourse.tile as tile
from concourse import bacc, mybir
from concourse import bass_utils

F32 = mybir.dt.float32
F16 = mybir.dt.float16
AF = mybir.ActivationFunctionType

# Hardcoded problem geometry (per-core shard)
D = 256      # feature dim (2 partition chunks of 128)
S = 512      # source positions (free axis)
TC = 256     # target positions per core
NCORES = 8
G = 16       # targets per ACT batch
USE_GPSIMD = True   # offload half the broadcast-adds to GPSIMD

_nc_cache = None


def build_nc():
    nc = bacc.Bacc("TRN2", target_bir_lowering=False, debug=False)

    slt = nc.dram_tensor("slt", [D, S], F32, kind="ExternalInput").ap()
    tlt = nc.dram_tensor("tlt", [D, TC], F32, kind="ExternalInput").ap()
    wbig = nc.dram_tensor("wbig", [D, 255], F16, kind="ExternalInput").ap()
    brow = nc.dram_tensor("brow", [1, 128], F16, kind="ExternalInput").ap()
    ones = nc.dram_tensor("ones", [1, S], F16, kind="ExternalInput").ap()
    genp = nc.dram_tensor("genp", [TC, S], F32, kind="ExternalOutput").ap()

    with tile.TileContext(nc) as tc, ExitStack() as ctx:
        const = ctx.enter_context(tc.tile_pool(name="const", bufs=1))
        sums = ctx.enter_context(tc.tile_pool(name="sums", bufs=1))
        work = ctx.enter_context(tc.tile_pool(name="work", bufs=2))
        psump = ctx.enter_context(
            tc.tile_pool(name="psum", bufs=2, space="PSUM")
        )

        # dc0 inputs first (first ACT group needs them), spread across two
        # DMA queues so the loads overlap.
        sltt, tltt, wt = [None, None], [None, None], [None, None]
        for dc in range(2):
            t1 = const.tile([128, S], F32, tag=f"slt{dc}")
            nc.sync.dma_start(out=t1[:], in_=slt[dc * 128:(dc + 1) * 128, :])
            sltt[dc] = t1
            t2 = const.tile([128, TC], F32, tag=f"tlt{dc}")
            nc.gpsimd.dma_start(out=t2[:], in_=tlt[dc * 128:(dc + 1) * 128, :])
            tltt[dc] = t2
        for dc in range(2):
            t3 = const.tile([128, 255], F16, tag=f"w{dc}")
            nc.sync.dma_start(out=t3[:], in_=wbig[dc * 128:(dc + 1) * 128, :])
            wt[dc] = t3
        bt = const.tile([1, 128], F16, tag="brow")
        nc.gpsimd.dma_start(out=bt[:], in_=brow[:, :])
        ot = const.tile([1, S], F16, tag="ones")
        nc.gpsimd.dma_start(out=ot[:], in_=ones[:, :])

        n_blocks = TC // 128
        n_groups = 128 // G
        for tb in range(n_blocks):
            ps = psump.tile([128, S], F32)
            # b_res via rank-1 K=1 matmul; start=True clears the bank.
            nc.tensor.matmul(ps[:], lhsT=bt[:], rhs=ot[:],
                             start=True, stop=False)
            for g in range(n_groups):
                for dc in range(2):
                    sm = sums.tile([128, G * S], F32, tag=f"sum{dc}")
                    for j in range(G):
                        t = tb * 128 + g * G + j
                        eng = nc.vector
                        if USE_GPSIMD and (j % 2 == 1):
                            eng = nc.gpsimd
                        eng.tensor_scalar_add(
                            sm[:, j * S:(j + 1) * S],
                            sltt[dc][:],
                            tltt[dc][:, t:t + 1],
                        )
                    h = work.tile([128, G * S], F16, tag=f"h{dc}")
                    nc.scalar.activation(h[:], sm[:], AF.Tanh)
                    for j in range(G):
                        tl = g * G + j  # local row in this psum block
                        last = (g == n_groups - 1) and (j == G - 1) and (dc == 1)
                        nc.tensor.matmul(
                            ps[:],
                            lhsT=wt[dc][:, 127 - tl:255 - tl],
                            rhs=h[:, j * S:(j + 1) * S],
                            start=False,
                            stop=last,
                        )
            ob = work.tile([128, S], F32, tag="out")
            nc.vector.tensor_copy(ob[:], ps[:])
            nc.sync.dma_start(
                out=genp[tb * 128:(tb + 1) * 128, :], in_=ob[:]
            )

    nc.compile()
    return nc


def _get_nc():
    global _nc_cache
    if _nc_cache is None:
        _nc_cache = build_nc()
    return _nc_cache


def _host_prep(source, target, W_src, b_src, W_tgt, b_tgt, W_res, b_res):
    B = source.shape[0]
    T = target.shape[1]
    src_lin = source.reshape(B * S, D).astype(np.float32) @ W_src + b_src
    tgt_lin = target.reshape(B * T, D).astype(np.float32) @ W_tgt + b_tgt
    src_linT = np.ascontiguousarray(
        src_lin.reshape(B, S, D).transpose(0, 2, 1)
    )  # (B, D, S)
    tgt_linT = np.ascontiguousarray(
        tgt_lin.reshape(B, T, D).transpose(0, 2, 1)
    )  # (B, D, T)
    wbig = np.zeros((D, 255), np.float16)
    wbig[:, 127] = W_res[:, 0].astype(np.float16)
    brow = np.full((1, 128), np.float32(b_res[0]), np.float16)
    ones = np.ones((1, S), np.float16)
    in_maps = []
    for c in range(NCORES):
        b, th = c // 2, c % 2
        in_maps.append({
            "slt": src_linT[b],
            "tlt": np.ascontiguousarray(
                tgt_linT[b, :, th * TC:(th + 1) * TC]
            ),
            "wbig": wbig,
            "brow": brow,
            "ones": ones,
        })
    return in_maps


def kernel(source, target, W_src, b_src, W_tgt, b_tgt, W_res, b_res,
           W_prob, b_prob, _trace=False):
    source = np.asarray(source, np.float32)
    target = np.asarray(target, np.float32)
    W_src = np.asarray(W_src, np.float32)
    b_src = np.asarray(b_src, np.float32)
    W_tgt = np.asarray(W_tgt, np.float32)
    b_tgt = np.asarray(b_tgt, np.float32)
    W_res = np.asarray(W_res, np.float32)
    b_res = np.asarray(b_res, np.float32)
    W_prob = np.asarray(W_prob, np.float32)
    b_prob = np.asarray(b_prob, np.float32)

    B = source.shape[0]
    T = target.shape[1]

    in_maps = _host_prep(source, target, W_src, b_src, W_tgt, b_tgt,
                         W_res, b_res)
    nc = _get_nc()
    if not _trace:
        # The axon NTFF trace path needs antenv.axon_hooks, which this
        # image lacks; make sure an inherited BASS_TRACE can't divert us.
        os.environ["BASS_NEVER_TRACE"] = "1"
    else:
        os.environ.pop("BASS_NEVER_TRACE", None)
    res = bass_utils.run_bass_kernel_spmd(
        nc, in_maps, list(range(NCORES)), trace=_trace
    )

    genP = np.empty((B, T, S), np.float32)
    for c in range(NCORES):
        b, th = c // 2, c % 2
        genP[b, th * TC:(th + 1) * TC, :] = res.results[c]["genp"]

    logits = target.reshape(B * T, D) @ W_prob + b_prob
    m = logits.max(axis=-1, keepdims=True)
    e = np.exp(logits - m)
    prob = (e / e.sum(axis=-1, keepdims=True)).reshape(B, T, 2)
    prob = prob.astype(np.float32)

    if _trace:
        kernel._last_result = res
    return genP, prob


# revision 13
# speedup vs baseline: 1.1563x; 1.0734x over previous
"""CopyNet additive-attention kernel for 8 Trainium2 NeuronCores.

Problem (B=4, S=T=512, D=256):
    src_lin = source @ W_src + b_src                    (B,S,D)
    tgt_lin = target @ W_tgt + b_tgt                    (B,T,D)
    genP    = tanh(src_lin[:,None] + tgt_lin[:,:,None]) @ W_res[:,0] + b_res
    prob    = softmax(target @ W_prob + b_prob)         (B,T,2)

The O(B*T*S*D) = 268M-element tanh + weighted reduction runs on device; the
tiny linear layers / prob head (0.2% of the FLOPs) are host-side prep.

Sharding: 8 cores = 4 batches x 2 target-halves. Each core handles one
(b, 256-target-slice) shard: 33.5M tanh evaluations.

Per-core device program (D on partitions as 2x128 chunks, S on free axis),
all in fp16 around the tanh (absmax-rel ~3e-4):
  - DVE : broadcast-add tgt_lin[:,t] (per-partition scalar). fp16 SBUF
          operands engage the 4x_2P perf mode -> ~194ns per target.
  - ACT : tanh over 16-target batches (amortizes the ~224-cycle per-instr
          overhead); fp16 in -> fp16 out; the bottleneck engine at
          1 elem/cycle/lane/1.2GHz -> ~224us busy per core.
  - PE  : fp16 matmuls (1 cyc/row) with a shifted-diagonal weight tile so
          each target's D-reduction accumulates into its own PSUM partition
          row; b_res enters as a K=1 rank-1 matmul.
"""

import os
import numpy as np
from contextlib import ExitStack

import concourse.bass as bass
import concourse.tile as tile
from concourse import bacc, mybir
from concourse import bass_utils

F32 = mybir.dt.float32
F16 = mybir.dt.float16
AF = mybir.ActivationFunctionType

# Hardcoded problem geometry (per-core shard)
D = 256      # feature dim (2 partition chunks of 128)
S = 512      # source positions (free axis)
TC = 256     # target positions per core
NCORES = 8
G = 16       # targets per ACT batch

_nc_cache = None


def build_nc():
    nc = bacc.Bacc("TRN2", target_bir_lowering=False, debug=False)

    slt = nc.dram_tensor("slt", [D, S], F16, kind="ExternalInput").ap()
    # tlt stays fp32: tensor_scalar's per-partition scalar operand must be
    # fp32 (and is exempt from the 4x-mode dtype checks).
    tlt = nc.dram_tensor("tlt", [D, TC], F32, kind="ExternalInput").ap()
    wbig = nc.dram_tensor("wbig", [D, 255], F16, kind="ExternalInput").ap()
    brow = nc.dram_tensor("brow", [1, 128], F16, kind="ExternalInput").ap()
    ones = nc.dram_tensor("ones", [1, S], F16, kind="ExternalInput").ap()
    genp = nc.dram_tensor("genp", [TC, S], F32, kind="ExternalOutput").ap()

    with tile.TileContext(nc) as tc, ExitStack() as ctx:
        const = ctx.enter_context(tc.tile_pool(name="const", bufs=1))
        sums = ctx.enter_context(tc.tile_pool(name="sums", bufs=2))
        work = ctx.enter_context(tc.tile_pool(name="work", bufs=2))
        psump = ctx.enter_context(
            tc.tile_pool(name="psum", bufs=2, space="PSUM")
        )

        # dc0 inputs first (first ACT group needs them), spread across two
        # DMA queues so the loads overlap.
        sltt, tltt, wt = [None, None], [None, None], [None, None]
        for dc in range(2):
            t1 = const.tile([128, S], F16, tag=f"slt{dc}")
            nc.sync.dma_start(out=t1[:], in_=slt[dc * 128:(dc + 1) * 128, :])
            sltt[dc] = t1
            t2 = const.tile([128, TC], F32, tag=f"tlt{dc}")
            nc.gpsimd.dma_start(out=t2[:], in_=tlt[dc * 128:(dc + 1) * 128, :])
            tltt[dc] = t2
        for dc in range(2):
            t3 = const.tile([128, 255], F16, tag=f"w{dc}")
            nc.sync.dma_start(out=t3[:], in_=wbig[dc * 128:(dc + 1) * 128, :])
            wt[dc] = t3
        bt = const.tile([1, 128], F16, tag="brow")
        nc.gpsimd.dma_start(out=bt[:], in_=brow[:, :])
        ot = const.tile([1, S], F16, tag="ones")
        nc.gpsimd.dma_start(out=ot[:], in_=ones[:, :])

        n_blocks = TC // 128
        n_groups = 128 // G
        for tb in range(n_blocks):
            ps = psump.tile([128, S], F32)
            # b_res via rank-1 K=1 matmul; start=True clears the bank.
            nc.tensor.matmul(ps[:], lhsT=bt[:], rhs=ot[:],
                             start=True, stop=False)
            for g in range(n_groups):
                for dc in range(2):
                    sm = sums.tile([128, G * S], F16, tag=f"sum{dc}")
                    for j in range(G):
                        t = tb * 128 + g * G + j
                        nc.vector.tensor_scalar_add(
                            sm[:, j * S:(j + 1) * S],
                            sltt[dc][:],
                            tltt[dc][:, t:t + 1],
                        )
                    h = work.tile([128, G * S], F16, tag=f"h{dc}")
                    if tb == 0 and g == 0 and dc == 0:
                        # Prologue: split the first tanh batch so ACT can
                        # start as soon as the first few sums exist.
                        for sub in range(4):
                            sl = slice(sub * 4 * S, (sub + 1) * 4 * S)
                            nc.scalar.activation(h[:, sl], sm[:, sl], AF.Tanh)
                    else:
                        nc.scalar.activation(h[:], sm[:], AF.Tanh)
                    for j in range(G):
                        tl = g * G + j  # local row in this psum block
                        last = (g == n_groups - 1) and (j == G - 1) and (dc == 1)
                        nc.tensor.matmul(
                            ps[:],
                            lhsT=wt[dc][:, 127 - tl:255 - tl],
                            rhs=h[:, j * S:(j + 1) * S],
                            start=False,
                            stop=last,
                        )
            ob = work.tile([128, S], F32, tag="out")
            nc.vector.tensor_copy(ob[:], ps[:])
            nc.sync.dma_start(
                out=genp[tb * 128:(tb + 1) * 128, :], in_=ob[:]
            )

    nc.compile()
    return nc


def _get_nc():
    global _nc_cache
    if _nc_cache is None:
        _nc_cache = build_nc()
    return _nc_cache


def _host_prep(source, target, W_src, b_src, W_tgt, b_tgt, W_res, b_res):
    B = source.shape[0]
    T = target.shape[1]
    src_lin = source.reshape(B * S, D).astype(np.float32) @ W_src + b_src
    tgt_lin = target.reshape(B * T, D).astype(np.float32) @ W_tgt + b_tgt
    src_linT = np.ascontiguousarray(
        src_lin.reshape(B, S, D).transpose(0, 2, 1).astype(np.float16)
    )  # (B, D, S)
    tgt_linT = np.ascontiguousarray(
        tgt_lin.reshape(B, T, D).transpose(0, 2, 1).astype(np.float32)
    )  # (B, D, T)
    wbig = np.zeros((D, 255), np.float16)
    wbig[:, 127] = W_res[:, 0].astype(np.float16)
    brow = np.full((1, 128), np.float32(b_res[0]), np.float16)
    ones = np.ones((1, S), np.float16)
    in_maps = []
    for c in range(NCORES):
        b, th = c // 2, c % 2
        in_maps.append({
            "slt": src_linT[b],
            "tlt": np.ascontiguousarray(
                tgt_linT[b, :, th * TC:(th + 1) * TC]
            ),
            "wbig": wbig,
            "brow": brow,
            "ones": ones,
        })
    return in_maps


def kernel(source, target, W_src, b_src, W_tgt, b_tgt, W_res, b_res,
           W_prob, b_prob, _trace=False):
    source = np.asarray(source, np.float32)
    target = np.asarray(target, np.float32)
    W_src = np.asarray(W_src, np.float32)
    b_src = np.asarray(b_src, np.float32)
    W_tgt = np.asarray(W_tgt, np.float32)
    b_tgt = np.asarray(b_tgt, np.float32)
    W_res = np.asarray(W_res, np.float32)
    b_res = np.asarray(b_res, np.float32)
    W_prob = np.asarray(W_prob, np.float32)
    b_prob = np.asarray(b_prob, np.float32)

    B = source.shape[0]
    T = target.shape[1]

    in_maps = _host_prep(source, target, W_src, b_src, W_tgt, b_tgt,
                         W_res, b_res)
    nc = _get_nc()
    if not _trace:
        # The axon NTFF trace path needs antenv.axon_hooks, which this
        # image lacks; make sure an inherited BASS_TRACE can't divert us.
        os.environ["BASS_NEVER_TRACE"] = "1"
    else:
        os.environ.pop("BASS_NEVER_TRACE", None)
    res = bass_utils.run_bass_kernel_spmd(
        nc, in_maps, list(range(NCORES)), trace=_trace
    )

    genP = np.empty((B, T, S), np.float32)
    for c in range(NCORES):
        b, th = c // 2, c % 2
        genP[b, th * TC:(th + 1) * TC, :] = res.results[c]["genp"]

    logits = target.reshape(B * T, D) @ W_prob + b_prob
    m = logits.max(axis=-1, keepdims=True)
    e = np.exp(logits - m)
    prob = (e / e.sum(axis=-1, keepdims=True)).reshape(B, T, 2)
    prob = prob.astype(np.float32)

    if _trace:
        kernel._last_result = res
    return genP, prob
